# revision 44
# baseline (speedup 1.0000x reference)
"""Trainium2 8-core kernel for the GCN-encoder similarity problem.

Math (reference, simplified):
  A_hat = D^-1/2 (A + I) D^-1/2          (dense normalized adjacency, N x N)
  x1 = relu(A_hat @ (feat @ W1) + b1)
  x2 = A_hat @ (x1 @ W2) + b2
  sim = x2 @ x2.T
  out = sigmoid(softmax_rows(sim))       (pos_w1 row-scaling cancels in softmax)

Sharding: nodes split 8 ways (1024 rows/core).  Each core computes its
row-slice of every intermediate; AllGathers stitch the full y1/y2/x2
needed for the A_hat matmuls and the NxN similarity.  The adjacency
and similarity matmuls run in fp8e4 with DoubleRow (K=256/matmul);
fp32 accumulate throughout.  sigmoid(p) with p<1e-3 is evaluated as
0.5 + p/4 (error < p^3/48 ~ 1e-11, far below fp32 noise).  The output
is stored fp16 on device (quantization <= 2.5e-4 on ~0.5-magnitude
values, halving the dominant 32 MiB/core store stream) and widened to
fp32 on the host during unsharding; end-to-end error vs the fp32
reference is ~2.9e-4 absmax (~5.8e-4 relative).
"""
import sys
from contextlib import ExitStack

sys.path.insert(0, "/opt/trn_rl_repo")

import numpy as np
import ml_dtypes

import concourse.bacc as bacc
import concourse.mybir as mybir
import concourse.tile as tile
from concourse.bass_utils import run_bass_kernel_spmd

N = 8192
E = 131072
CIN = 512   # input feature dim
H = 512     # hidden dim (2 * OUT_C)
C2 = 256    # OUT_C
NCORES = 8
R = N // NCORES  # 1024 rows per core

BF16 = mybir.dt.bfloat16
F32 = mybir.dt.float32
F8 = mybir.dt.float8e4
F16 = mybir.dt.float16
bf16 = ml_dtypes.bfloat16
f8e4 = ml_dtypes.float8_e4m3

DR = mybir.MatmulPerfMode.DoubleRow

_BUILD_CACHE = {}


def _build(stub_ag=False):
    nc = bacc.Bacc(None, target_bir_lowering=False, debug=False)

    featT = nc.declare_dram_parameter("featT", [CIN, R], BF16, isOutput=False)
    W1d = nc.declare_dram_parameter("W1", [CIN, H], BF16, isOutput=False)
    W2d = nc.declare_dram_parameter("W2", [H, C2], BF16, isOutput=False)
    b1d = nc.declare_dram_parameter("b1", [H, 1], F32, isOutput=False)
    b2d = nc.declare_dram_parameter("b2", [C2, 1], F32, isOutput=False)
    ATs = nc.declare_dram_parameter("ATs", [N, R], F8, isOutput=False)
    outd = nc.declare_dram_parameter("out", [R, N], F16, isOutput=True)

    ag1_in = nc.dram_tensor("ag1_in", [R, H], F8)
    ag1_out = nc.dram_tensor("ag1_out", [N, H], F8, addr_space="Shared")
    ag2_in = nc.dram_tensor("ag2_in", [R, C2], F8)
    ag2_out = nc.dram_tensor("ag2_out", [N, C2], F8, addr_space="Shared")
    ag3_in = nc.dram_tensor("ag3_in", [C2, R], F8)
    ag3_out = nc.dram_tensor("ag3_out", [NCORES * C2, R], F8, addr_space="Shared")
    RG = [list(range(NCORES))]

    def gather(ag_i, ag_o):
        if stub_ag:
            nc.sync.dma_start(ag_o[0:ag_i.shape[0], :], ag_i[:, :])
        else:
            nc.gpsimd.collective_compute(
                "AllGather", mybir.AluOpType.bypass, replica_groups=RG,
                ins=[ag_i.ap().opt()], outs=[ag_o.ap().opt()],
            )

    AF = mybir.ActivationFunctionType
    with tile.TileContext(nc) as tc:
        with (
            tc.tile_pool(name="persist", bufs=1) as pb,
            tc.tile_pool(name="work", bufs=3) as wp,
        ):
            b1t = pb.tile([128, 4], F32)
            nc.sync.dma_start(b1t[:], b1d.rearrange("(a p) o -> p (a o)", p=128))
            b2t = pb.tile([128, 2], F32)
            nc.sync.dma_start(b2t[:], b2d.rearrange("(a p) o -> p (a o)", p=128))
            x2T = pb.tile([128, 2, R], F8)

            _atp_es = ExitStack()
            atp = _atp_es.enter_context(tc.tile_pool(name="atp", bufs=1))
            atsb = atp.tile([128, 64, R], F8)

            with tc.tile_pool(name="ph123", bufs=1) as pa:
                # ---- Phase 1: y1_slice = feat_slice @ W1  -> AllGather (fp8)
                ftile = pa.tile([128, 4, R], BF16)
                w1t = pa.tile([128, 4, H], BF16)
                for k in range(4):
                    nc.sync.dma_start(ftile[:, k, :], featT[k * 128:(k + 1) * 128, :])
                    nc.sync.dma_start(w1t[:, k, :], W1d[k * 128:(k + 1) * 128, :])
                # resident A_hat^T slice prefetch (needed from phase 2 on).
                # First half issued here (overlaps phase 1); the rest is issued
                # after the gather so the y1 loads are not queued behind it.
                for jc in range(4):
                    nc.sync.dma_start(
                        atsb[:, jc * 8:(jc + 1) * 8, :],
                        ATs[jc * 1024:(jc + 1) * 1024, :]
                        .rearrange("(a p) c -> p a c", p=128),
                    )
                with tc.tile_pool(name="ps1", bufs=1, space="PSUM") as psum:
                    for m in range(8):
                        ps = psum.tile([128, H], F32, tag="p1", bufs=2)
                        for k in range(4):
                            nc.tensor.matmul(
                                ps[:], ftile[:, k, m * 128:(m + 1) * 128], w1t[:, k, :],
                                start=(k == 0), stop=(k == 3),
                            )
                        y1b = wp.tile([128, H], F8, tag="y1b")
                        nc.vector.tensor_copy(y1b[:], ps[:])
                        nc.sync.dma_start(ag1_in[m * 128:(m + 1) * 128, :], y1b[:])
                gather(ag1_in, ag1_out)
                for jc in range(4, 8):
                    nc.sync.dma_start(
                        atsb[:, jc * 8:(jc + 1) * 8, :],
                        ATs[jc * 1024:(jc + 1) * 1024, :]
                        .rearrange("(a p) c -> p a c", p=128),
                    )

                # ---- Phase 2: x1T = relu((A_hat @ y1)^T + b1)  [H, R] bf16
                # fp8 DoubleRow: K=256 per matmul (2 j-chunks)
                y1f = pa.tile([128, 64, H], F8)
                for jc in range(8):
                    nc.sync.dma_start(
                        y1f[:, jc * 8:(jc + 1) * 8, :],
                        ag1_out[jc * 1024:(jc + 1) * 1024, :]
                        .rearrange("(a p) h -> p a h", p=128),
                    )
                x1T = pa.tile([128, 4, R], BF16)
                w2t = pa.tile([128, 4, C2], BF16)
                nc.sync.dma_start(w2t[:], W2d.rearrange("(a p) h -> p a h", p=128))
                with tc.tile_pool(name="ps2", bufs=1, space="PSUM") as psum:
                    pss = [[psum.tile([128, 512], F32, tag=f"p2_{rc}_{f}",
                                      name=f"pss{rc}_{f}", bufs=1)
                            for f in range(4)] for rc in range(2)]
                    for j2 in range(32):
                        for f in range(4):
                            for rc in range(2):
                                mm = nc.tensor.matmul(
                                    pss[rc][f][:],
                                    y1f[:, 2 * j2:2 * j2 + 2, f * 128:(f + 1) * 128],
                                    atsb[:, 2 * j2:2 * j2 + 2, rc * 512:(rc + 1) * 512],
                                    start=(j2 == 0), stop=(j2 == 31),
                                    perf_mode=DR,
                                )
                                # rc=0/1 share the same stationary y1f slice;
                                # skip the redundant reload for rc=1
                                if rc == 1:
                                    mm.ins.ldweights = False
                    for rc in range(2):
                        for f in range(4):
                            nc.scalar.activation(
                                x1T[:, f, rc * 512:(rc + 1) * 512], pss[rc][f][:],
                                AF.Relu, bias=b1t[:, f:f + 1],
                            )
                # ---- Phase 3: y2 = x1 @ W2 -> AllGather (fp8)
                with tc.tile_pool(name="ps3", bufs=1, space="PSUM") as psum:
                    for m in range(8):
                        ps3 = psum.tile([128, C2], F32, tag="p3", bufs=2)
                        for f in range(4):
                            nc.tensor.matmul(
                                ps3[:], x1T[:, f, m * 128:(m + 1) * 128], w2t[:, f, :],
                                start=(f == 0), stop=(f == 3),
                            )
                        y2b = wp.tile([128, C2], F8, tag="y2b")
                        nc.vector.tensor_copy(y2b[:], ps3[:])
                        nc.sync.dma_start(ag2_in[m * 128:(m + 1) * 128, :], y2b[:])
                gather(ag2_in, ag2_out)

            # ---- Phase 4: x2T = (A_hat @ y2)^T + b2  [C2, R] fp8 -> AllGather
            with (
                tc.tile_pool(name="ph4", bufs=1) as pc,
                tc.tile_pool(name="psB", bufs=1, space="PSUM") as psum,
            ):
                y2f = pc.tile([128, 64, C2], F8)
                for jc in range(8):
                    nc.sync.dma_start(
                        y2f[:, jc * 8:(jc + 1) * 8, :],
                        ag2_out[jc * 1024:(jc + 1) * 1024, :]
                        .rearrange("(a p) h -> p a h", p=128),
                    )
                ps4 = [[psum.tile([128, 512], F32, tag=f"p4_{rc}_{oc}",
                                  name=f"ps4_{rc}_{oc}", bufs=1)
                        for oc in range(2)] for rc in range(2)]
                for j2 in range(32):
                    for oc in range(2):
                        for rc in range(2):
                            mm = nc.tensor.matmul(
                                ps4[rc][oc][:],
                                y2f[:, 2 * j2:2 * j2 + 2, oc * 128:(oc + 1) * 128],
                                atsb[:, 2 * j2:2 * j2 + 2, rc * 512:(rc + 1) * 512],
                                start=(j2 == 0), stop=(j2 == 31),
                                perf_mode=DR,
                            )
                            # rc=0/1 share the same stationary y2f slice;
                            # skip the redundant reload for rc=1
                            if rc == 1:
                                mm.ins.ldweights = False
                for rc in range(2):
                    for oc in range(2):
                        nc.scalar.activation(
                            x2T[:, oc, rc * 512:(rc + 1) * 512], ps4[rc][oc][:],
                            AF.Identity, bias=b2t[:, oc:oc + 1],
                        )
                        nc.sync.dma_start(
                            ag3_in[oc * 128:(oc + 1) * 128, rc * 512:(rc + 1) * 512],
                            x2T[:, oc, rc * 512:(rc + 1) * 512],
                        )
            gather(ag3_in, ag3_out)
            _atp_es.close()

            # ---- Phase 5: sim rows + softmax + sigmoid-approx, streamed out
            with (
                tc.tile_pool(name="psC", bufs=4, space="PSUM") as psum,
                tc.tile_pool(name="ph5", bufs=3) as ep,
            ):
                x2a = pb.tile([128, 16, R], F8)
                for jc in range(8):
                    nc.sync.dma_start(
                        x2a[:, jc * 2:(jc + 1) * 2, :],
                        ag3_out[jc * 256:(jc + 1) * 256, :]
                        .rearrange("(a p) r -> p a r", p=128),
                    )
                for m in range(8):
                    acc = wp.tile([128, 4], F32, tag="acc")
                    e = ep.tile([128, 4, 2048], BF16, tag="e")
                    for g in range(4):
                        ps5 = psum.tile([128, 2048], F32, tag="p5", bufs=2)
                        for q in range(4):
                            cc = g * 4 + q
                            rb = cc // 2
                            wo = (cc % 2) * 512
                            mm = nc.tensor.matmul(
                                ps5[:, q * 512:(q + 1) * 512],
                                x2T[:, :, m * 128:(m + 1) * 128],
                                x2a[:, 2 * rb:2 * rb + 2, wo:wo + 512],
                                start=True, stop=True,
                                perf_mode=DR,
                            )
                            # All 16 matmuls of this row-block share the same
                            # stationary x2T slice; skip reloading it after
                            # the first (LDWEIGHTS elision, unmodeled in the
                            # cost model but real on hardware).
                            if g != 0 or q != 0:
                                mm.ins.ldweights = False
                        nc.scalar.activation(
                            e[:, g, :], ps5[:], AF.Exp, accum_out=acc[:, g:g + 1],
                        )
                    S = wp.tile([128, 1], F32, tag="S")
                    nc.vector.reduce_sum(S[:], acc[:], axis=mybir.AxisListType.X)
                    rS = wp.tile([128, 1], F32, tag="rS")
                    nc.vector.reciprocal(rS[:], S[:])
                    rS4 = wp.tile([128, 1], F32, tag="rS4")
                    nc.vector.tensor_scalar_mul(rS4[:], rS[:], 0.25)
                    o = ep.tile([128, N], F16, tag="o")
                    for g in range(4):
                        # alternate scale-and-bias chunks between DVE and
                        # GPSIMD so the output stores launch sooner
                        eng = nc.vector if g % 2 == 0 else nc.gpsimd
                        eng.tensor_scalar(
                            o[:, g * 2048:(g + 1) * 2048], e[:, g, :], rS4[:], 0.5,
                            op0=mybir.AluOpType.mult, op1=mybir.AluOpType.add,
                        )
                        nc.sync.dma_start(
                            outd[m * 128:(m + 1) * 128, g * 2048:(g + 1) * 2048],
                            o[:, g * 2048:(g + 1) * 2048],
                        )
    nc.compile()
    return nc


def _get_nc():
    if "nc" not in _BUILD_CACHE:
        _BUILD_CACHE["nc"] = _build()
    return _BUILD_CACHE["nc"]


def _prep_inputs(feat, edge_index, W1, b1, W2, b2):
    feat = np.asarray(feat, np.float32)
    ei = np.asarray(edge_index).astype(np.int64)
    row = np.concatenate([ei[0], np.arange(N, dtype=np.int64)])
    col = np.concatenate([ei[1], np.arange(N, dtype=np.int64)])
    deg = np.bincount(col, minlength=N).astype(np.float32)
    dinv = np.where(deg > 0, 1.0 / np.sqrt(deg), 0.0).astype(np.float32)
    # AT[j, i] = A_hat[i, j] (source j, destination i)
    AT = np.zeros((N, N), np.float32)
    np.add.at(AT, (row, col), dinv[row] * dinv[col])
    AT = AT.astype(f8e4)

    W1b = np.ascontiguousarray(np.asarray(W1, np.float32)).astype(bf16)
    W2b = np.ascontiguousarray(np.asarray(W2, np.float32)).astype(bf16)
    b1c = np.ascontiguousarray(np.asarray(b1, np.float32).reshape(H, 1))
    b2c = np.ascontiguousarray(np.asarray(b2, np.float32).reshape(C2, 1))
    featb = feat.astype(bf16)

    in_maps = []
    for c in range(NCORES):
        sl = slice(c * R, (c + 1) * R)
        in_maps.append({
            "featT": np.ascontiguousarray(featb[sl].T),
            "W1": W1b,
            "W2": W2b,
            "b1": b1c,
            "b2": b2c,
            "ATs": np.ascontiguousarray(AT[:, sl]),
        })
    return in_maps


def kernel(feat, edge_index, W1, b1, W2, b2, W3=None, b3=None, _trace=False):
    nc = _get_nc()
    in_maps = _prep_inputs(feat, edge_index, W1, b1, W2, b2)
    res = run_bass_kernel_spmd(
        nc, in_maps, core_ids=list(range(NCORES)), trace=_trace,
    )
    out = np.concatenate(
        [res.results[c]["out"].astype(np.float32) for c in range(NCORES)], axis=0)
    if _trace:
        kernel.last_results = res
    return out
